# revision 47
# baseline (speedup 1.0000x reference)
"""Multi-head attention (B=2, S=2048, D=1024, H=16) on 8 Trainium2 cores.

Sharding: data-parallel over the 2 batches x tensor-parallel over 4 groups
of 4 heads.  Core c handles batch c//4 and heads [4*(c%4) : 4*(c%4)+4]
(columns [256*(c%4) : +256] of Wk/Wv, same rows of Wo).  Each core produces
a partial [S, D] output (its heads' contribution to o @ Wo); the host sums
the 4 partials per batch (and adds bo once).

Per-core dataflow (bf16 everywhere; fp32 PSUM accumulation):
  qT,kT,vT [D,S] arrive pre-transposed AND pre-cast to bf16 on the host, so
  all loads ride the fast HWDGE queues at half the bytes.  Projections
  produce QT,KT [128,2,S] (head-major rows) and V [sk,hd] with an extra
  ones column.  Attention per head in "scores-transposed" layout
  [sk_part, sq_free]: scoresT = KT_j^T @ QT (one matmul per k-block row);
  exp on ScalarE (scale folded in; no max subtraction - scores are O(6));
  the causal diagonal block is handled by zeroing the upper triangle of the
  exp tile with one DVE multiply (cheaper than the identity-matmul + mask
  trick: no PE work, no ident reloads).  UT[65, S] += Vaug_j^T @ expT
  accumulated in PSUM, row 64 = softmax denominators (from the ones
  column).  Normalization is region-wise (512 cols at a time, as soon as
  that region's last k-block lands): sums -> DMA reshape [1,512]->[128,4]
  -> DVE reciprocal -> DMA back -> gpsimd partition_broadcast -> one DVE
  multiply into oT [d_part, sq].  Final: out = oT^T @ Wo per 128-row
  block, bf16 DMA to HBM (host sums partials in fp32).

A few throwaway warm-up matmuls run during the initial DMA fill so the PE
HAM clock-gate is already at 2.4 GHz when the first projection issues.
"""

from contextlib import ExitStack

import numpy as np

import concourse.bass as bass
import concourse.tile as tile
from concourse import bacc, bass_utils, mybir
from concourse.masks import make_identity

B, S, D, H = 2, 2048, 1024, 16
HD = D // H            # 64
NCORES = 8
HPC = 4                # heads per core
CW = HPC * HD          # 256 weight cols per core
NCH = 4                # sequence chunks of 512
S_INV = float(1.0 / (np.sqrt(np.float32(HD)) + np.float32(1e-8)))

F32 = mybir.dt.float32
BF16 = mybir.dt.bfloat16


def _build(mode: str, bias_k: bool, bias_v: bool):
    """Build + compile the SPMD program.  mode: 'causal' | 'none' | 'general'"""
    nc = bacc.Bacc("TRN2", target_bir_lowering=False, debug=False,
                   num_devices=NCORES)

    # q/k/v arrive host-pre-tiled as [NCH*128, 8*512]: row c*128+p, col
    # dc*512+s holds xT[dc*128+p, c*512+s].  One chunk = one DMA with 8KB
    # contiguous per-partition lines (vs 1KB slicing [D,S] directly).
    qT_d = nc.dram_tensor("qT", [NCH * 128, 8 * 512], BF16, kind="ExternalInput").ap()
    kT_d = nc.dram_tensor("kT", [NCH * 128, 8 * 512], BF16, kind="ExternalInput").ap()
    vT_d = nc.dram_tensor("vT", [NCH * 128, 8 * 512], BF16, kind="ExternalInput").ap()
    wk_d = nc.dram_tensor("wk", [D, CW], BF16, kind="ExternalInput").ap()
    wv_d = nc.dram_tensor("wv", [D, CW], BF16, kind="ExternalInput").ap()
    wo_d = nc.dram_tensor("wo", [CW, D], BF16, kind="ExternalInput").ap()
    bk_d = nc.dram_tensor("bk", [1, CW], BF16, kind="ExternalInput").ap() if bias_k else None
    bv_d = nc.dram_tensor("bv", [1, CW], BF16, kind="ExternalInput").ap() if bias_v else None
    maskT_d = (nc.dram_tensor("maskT", [S, S], BF16, kind="ExternalInput").ap()
               if mode == "general" else None)
    ones1_d = (nc.dram_tensor("ones1", [1, 512], BF16, kind="ExternalInput").ap()
               if (bias_k or bias_v) else None)
    out_d = nc.dram_tensor("out", [S, D], BF16, kind="ExternalOutput").ap()

    with tile.TileContext(nc) as tc, ExitStack() as ctx:
        sb1 = ctx.enter_context(tc.tile_pool(name="persist", bufs=1))
        v_pool = ctx.enter_context(tc.tile_pool(name="v", bufs=NCH))
        stage_pool = ctx.enter_context(tc.tile_pool(name="stage", bufs=6))
        exp_pool = ctx.enter_context(tc.tile_pool(name="exp", bufs=4))
        sums_pool = ctx.enter_context(tc.tile_pool(name="sums", bufs=4))
        srt_pool = ctx.enter_context(tc.tile_pool(name="srt", bufs=4))
        rcb_pool = ctx.enter_context(tc.tile_pool(name="rcb", bufs=4))
        bc_pool = ctx.enter_context(tc.tile_pool(name="bc", bufs=5))
        u_pool = ctx.enter_context(tc.tile_pool(name="u", bufs=4))
        ottmp_pool = ctx.enter_context(tc.tile_pool(name="ottmp", bufs=2))
        outsb_pool = ctx.enter_context(tc.tile_pool(name="outsb", bufs=4))
        sc_pool = ctx.enter_context(tc.tile_pool(name="sc", bufs=3, space="PSUM"))
        ut_pool = ctx.enter_context(tc.tile_pool(name="ut", bufs=1, space="PSUM"))
        if mode == "general":
            mask_pool = ctx.enter_context(tc.tile_pool(name="mask", bufs=3))

        # ---- PE warm-up: keep HAM busy while the first inputs stream in.
        # The seed memset rides DVE: GpSimd's wake-up path would hold the
        # first matmul until ~10us.  Results are discarded.
        wm = sb1.tile([128, 512], BF16)
        nc.vector.memset(wm[:], 0.0)
        wps = sc_pool.tile([128, 512], F32, tag="sc", name="warm")
        NWARM = 20
        for i in range(NWARM):
            nc.tensor.matmul(wps[:], wm[:, 0:128], wm[:],
                             start=(i == 0), stop=(i == NWARM - 1))

        # ---- constants / weights -------------------------------------
        # weights ride the Scalar HWDGE ring so the first k/q chunk leads
        # the Sync ring - the two rings ramp in parallel at kernel start
        wk_sb = sb1.tile([128, 8, CW], BF16)
        wv_sb = sb1.tile([128, 8, CW], BF16)
        nc.scalar.dma_start(wk_sb[:], wk_d.rearrange("(c p) n -> p c n", p=128))
        nc.scalar.dma_start(wv_sb[:], wv_d.rearrange("(c p) n -> p c n", p=128))
        wo_sb = sb1.tile([128, 2, D], BF16)  # loaded late, after staging
        if bias_k:
            bk_sb = sb1.tile([1, CW], BF16)
            nc.scalar.dma_start(bk_sb[:], bk_d[:])
        if bias_v:
            bv_sb = sb1.tile([1, CW], BF16)
            nc.scalar.dma_start(bv_sb[:], bv_d[:])
        if bias_k or bias_v:
            ones_sb = sb1.tile([1, 512], BF16)
            nc.scalar.dma_start(ones_sb[:], ones1_d[:])
        if mode == "general":
            ident = sb1.tile([128, 128], BF16)
            make_identity(nc, ident[:])
        if mode == "causal":
            # trimask[p, f] = 1 where f >= p (sq >= sk), else 0
            trimask = sb1.tile([128, 128], BF16)
            nc.gpsimd.memset(trimask[:], 1.0)
            nc.gpsimd.affine_select(
                out=trimask[:], in_=trimask[:],
                compare_op=mybir.AluOpType.is_ge,
                fill=0.0, base=0,
                pattern=[[1, 128]], channel_multiplier=-1,
            )

        # V tiles: [128 sk, 4 blk, 4 head, 66] - col 64 is the ones column
        v_tiles = [v_pool.tile([128, 4, HPC, 66], BF16, tag="v", name=f"v{c}")
                   for c in range(NCH)]
        for c in range(NCH):
            nc.gpsimd.memset(v_tiles[c][:, :, :, 64:65], 1.0)
        qt_sb = sb1.tile([128, 2, S], BF16)
        kt_sb = sb1.tile([128, 2, S], BF16)
        oT_sb = sb1.tile([128, 2, S], BF16)

        def ps_copy(dst, src):
            nc.vector.tensor_copy(dst, src)

        # ---- phase 1: projections (helpers) --------------------------
        def load_stage(c, nm, td, split=False):
            stg = stage_pool.tile([128, 8, 512], BF16, tag="stage",
                                  name=f"{nm}st{c}")
            src = td[bass.ds(128 * c, 128), :].rearrange(
                "p (dc s) -> p dc s", dc=8)
            if split:
                # split so the consumer can begin on the first half
                nc.sync.dma_start(stg[:, 0:4, :], src[:, 0:4, :])
                nc.sync.dma_start(stg[:, 4:8, :], src[:, 4:8, :])
            else:
                nc.sync.dma_start(stg[:], src)
            return stg

        def emit_proj_loads(c):
            # c >= 2 only; load in the order projections consume
            return [load_stage(c, nm, td) for nm, td in
                    (("q", qT_d), ("k", kT_d), ("v", vT_d))]

        def proj_mm_units(c, stages, part="both"):
            """Yield once per PSUM accumulation group (small PE work unit)."""
            if c >= 2:
                (qst, kst, vst) = stages
            else:
                (kst, qst, vst) = stages
            do_kq = part in ("both", "kq")
            do_v = part in ("both", "v")
            # KT / QT projections (transposed layout, 2 m-halves of 128)
            kq_order = ((((qst, qt_sb), (kst, kt_sb)) if c >= 2 else
                        ((kst, kt_sb), (qst, qt_sb))) if do_kq else ())
            for ti, (st, dst) in enumerate(kq_order):
                for m in range(2):
                    ps = sc_pool.tile([128, 512], F32, tag="sc", name=f"psp{c}_{ti}_{m}")
                    first = True
                    if bias_k:
                        nc.tensor.matmul(ps[:], bk_sb[0:1, bass.ds(m * 128, 128)],
                                         ones_sb[0:1, :], start=True, stop=False)
                        first = False
                    for dc in range(8):
                        nc.tensor.matmul(
                            ps[:],
                            wk_sb[:, dc, bass.ds(m * 128, 128)],
                            st[:, dc, :],
                            start=first, stop=(dc == 7))
                        first = False
                    ps_copy(dst[:, m, bass.ds(c * 512, 512)], ps[:])
                    yield
            # V projection (natural layout)
            for half in range(2) if do_v else ():
                psv = sc_pool.tile([128, 512], F32, tag="sc", name=f"psv{c}_{half}")
                for loc in range(2):
                    blk = 2 * half + loc
                    reg = psv[:, bass.ds(loc * 256, 256)]
                    first = True
                    if bias_v:
                        nc.tensor.matmul(reg, ones_sb[0:1, 0:128], bv_sb[0:1, :],
                                         start=True, stop=False)
                        first = False
                    for dc in range(8):
                        nc.tensor.matmul(
                            reg,
                            vst[:, dc, bass.ds(blk * 128, 128)],
                            wv_sb[:, dc, :],
                            start=first, stop=(dc == 7))
                        first = False
                ps_copy(v_tiles[c][:, bass.ds(2 * half, 2), :, 0:64],
                        psv[:].rearrange("p (b h e) -> p b h e", b=2, h=HPC))
                yield

        # ---- phase 2: attention, one (head, sq-half) pass ------------
        full_grid = mode != "causal"

        def attn_half(hl, half, tail_norm=False):
            m = hl // 2
            p0 = 64 * (hl % 2)
            base = 1024 * half
            regions = (2 * half, 2 * half + 1)
            ut = ut_pool.tile([128, 1024], F32, tag="ut", name=f"ut{hl}_{half}")

            if full_grid:
                steps = list(range(16))
                last_j = {r: 15 for r in regions}
            else:
                steps = list(range(8 * half + 8))
                last_j = {r: 4 * r + 3 for r in regions}

            win_ps = {}
            win_exp = {}

            def active(j):
                """absolute start col of k-block j's active window portion."""
                return base if full_grid else max(128 * j, base)

            def emit_scores(j):
                ps = sc_pool.tile([128, 1024], F32, tag="sc",
                                  name=f"sc{hl}_{half}_{j}")
                win_ps[j] = ps
                a0 = active(j)
                lhsT = kt_sb[p0:p0 + 64, m, bass.ds(128 * j, 128)]
                if mode == "causal":
                    for s in range(2):
                        lo, hi = base + 512 * s, base + 512 * s + 512
                        if hi <= a0:
                            continue
                        nlo = max(lo, a0)
                        nc.tensor.matmul(
                            ps[:, bass.ds(nlo - base, hi - nlo)],
                            lhsT,
                            qt_sb[p0:p0 + 64, m, bass.ds(nlo, hi - nlo)],
                            start=True, stop=True)
                    return
                if mode == "general":
                    mt = mask_pool.tile([128, 1024], BF16, tag="mask",
                                        name=f"mt{hl}_{half}_{j}")
                    nc.sync.dma_start(
                        mt[:],
                        maskT_d[bass.ds(128 * j, 128), bass.ds(base, 1024)])
                for s in range(2):
                    lo = base + 512 * s
                    reg = ps[:, bass.ds(lo - base, 512)]
                    rhs = qt_sb[p0:p0 + 64, m, bass.ds(lo, 512)]
                    nc.tensor.matmul(reg, lhsT, rhs, start=True,
                                     stop=(mode != "general"))
                    if mode == "general":
                        nc.tensor.matmul(reg, ident[:],
                                         mt[:, bass.ds(lo - base, 512)],
                                         start=False, stop=True)

            def emit_exp(j):
                ps = win_ps[j]
                off = active(j) - base
                et = exp_pool.tile([128, 1024], BF16, tag="exp",
                                   name=f"e{hl}_{half}_{j}")
                win_exp[j] = et
                nc.scalar.activation(et[:, off:1024], ps[:, off:1024],
                                     mybir.ActivationFunctionType.Exp, scale=S_INV)
                if mode == "causal" and 128 * j >= base:
                    # zero the upper triangle (sq < sk) of the diagonal block
                    # (DVE: GpSimd's ~1us semaphore-wait latency poisons the
                    # exp->PV chain)
                    d0 = 128 * j - base
                    nc.vector.tensor_mul(et[:, bass.ds(d0, 128)],
                                         et[:, bass.ds(d0, 128)], trimask[:])

            def emit_pv(j):
                et = win_exp.pop(j)
                win_ps.pop(j)
                a0 = active(j)
                for s in range(2):
                    lo, hi = base + 512 * s, base + 512 * s + 512
                    if hi <= a0:
                        continue
                    nlo = max(lo, a0)
                    r = nlo // 512
                    nc.tensor.matmul(
                        ut[0:65, bass.ds(nlo - base, hi - nlo)],
                        v_tiles[j // 4][:, j % 4, hl, 0:65],
                        et[:, bass.ds(nlo - base, hi - nlo)],
                        start=(j == 0), stop=(j == last_j[r]))

            if p0 == 0:
                dst = oT_sb[0:64, m, bass.ds(base, 1024)]
                ott = None
            else:
                ott = ottmp_pool.tile([64, 1024], BF16, tag="ottmp",
                                      name=f"ott{hl}_{half}")
                dst = ott[:, :]

            def emit_norm(r):
                """copy U+sums out of PSUM, then recip -> bcast -> multiply."""
                # the very last region's small DMAs ride the scalar ring:
                # the sync ring is busy draining output stores by then
                dma_eng = (nc.scalar if (tail_norm and r == regions[1])
                           else nc.sync)
                u = u_pool.tile([65, 512], F32, tag="u", name=f"u{hl}_{r}")
                nc.vector.tensor_copy(u[:], ut[0:65, bass.ds(512 * r - base, 512)])
                srt = srt_pool.tile([128, 4], F32, tag="srt", name=f"srt{hl}_{r}")
                dma_eng.dma_start(srt[:], u[64:65, :])
                nc.vector.reciprocal(srt[:], srt[:])
                rcb = rcb_pool.tile([1, 512], F32, tag="rcb", name=f"rcb{hl}_{r}")
                dma_eng.dma_start(rcb[0:1, :], srt[:])
                bc = bc_pool.tile([64, 512], F32, tag="bc", name=f"bc{hl}_{r}")
                nc.gpsimd.partition_broadcast(bc[:], rcb[:], channels=64)
                nc.vector.tensor_mul(
                    dst[:, bass.ds(512 * r - base, 512)],
                    u[0:64, :],
                    bc[:, :])
                if p0:
                    nc.sync.dma_start(
                        oT_sb[64:128, m, bass.ds(512 * r, 512)],
                        ott[:, bass.ds(512 * r - base, 512)])

            LOOKAHEAD = 2
            for i in range(min(LOOKAHEAD, len(steps))):
                emit_scores(steps[i])
            for i, j in enumerate(steps):
                if i + LOOKAHEAD < len(steps):
                    emit_scores(steps[i + LOOKAHEAD])
                emit_exp(j)
                emit_pv(j)
                for r in regions:
                    if j == last_j[r]:
                        emit_norm(r)
                yield

        def emit_final(sb, store_eng=None, copy_engs=None):
            ob = outsb_pool.tile([128, D], BF16, tag="outsb", name=f"ob{sb}")
            for nh in range(2):
                ps = sc_pool.tile([128, 512], F32, tag="sc", name=f"pso{sb}_{nh}")
                for mm_ in range(2):
                    nc.tensor.matmul(
                        ps[:],
                        oT_sb[:, mm_, bass.ds(sb * 128, 128)],
                        wo_sb[:, mm_, bass.ds(nh * 512, 512)],
                        start=(mm_ == 0), stop=(mm_ == 1))
                eng = (copy_engs or (nc.vector, nc.vector))[nh]
                if eng is nc.scalar:
                    nc.scalar.copy(ob[:, bass.ds(nh * 512, 512)], ps[:])
                else:
                    eng.tensor_copy(ob[:, bass.ds(nh * 512, 512)], ps[:])
            (store_eng or nc.sync).dma_start(out_d[bass.ds(sb * 128, 128), :], ob[:])

        def drain(gen):
            for _ in gen:
                pass

        def weave(step_gen, unit_gen, steps_per_unit):
            """Emit attention steps, inserting one PE-heavy unit every N."""
            i = 0
            for _ in step_gen:
                i += 1
                if i % steps_per_unit == 0:
                    next(unit_gen, None)
            for _ in unit_gen:
                pass

        def chain(*gens):
            for g in gens:
                yield from g

        # ---- orchestration: overlap proj DMA with attention ----------
        # chunk-0 k/q split + chunk-1 k hoisted before chunk-0 v: loads
        # land in the exact order projection 0/1 consume them.
        k0 = load_stage(0, "k", kT_d, split=True)
        q0 = load_stage(0, "q", qT_d, split=True)
        k1 = load_stage(1, "k", kT_d)
        v0 = load_stage(0, "v", vT_d)
        q1 = load_stage(1, "q", qT_d)
        v1 = load_stage(1, "v", vT_d)
        st0, st1 = (k0, q0, v0), (k1, q1, v1)
        drain(proj_mm_units(0, st0))
        st2 = emit_proj_loads(2)
        if full_grid:
            drain(proj_mm_units(1, st1))
            st3 = emit_proj_loads(3)
            nc.sync.dma_start(wo_sb[:], wo_d.rearrange("(m p) n -> p m n", p=128))
            # every k-block needs every chunk: project everything first
            drain(proj_mm_units(2, st2))
            drain(proj_mm_units(3, st3))
            for hl in (1, 3, 0, 2):
                drain(attn_half(hl, 0))
        else:
            drain(proj_mm_units(1, st1, part="kq"))
            half0s = chain(*[attn_half(hl, 0) for hl in (1, 3, 0, 2)])
            # a few attention steps cover v-chunk-1's DMA landing window
            # so its projection doesn't head-of-line-block the PE
            for _ in range(3):
                next(half0s, None)
            drain(proj_mm_units(1, st1, part="v"))
            st3 = emit_proj_loads(3)
            nc.sync.dma_start(wo_sb[:], wo_d.rearrange("(m p) n -> p m n", p=128))
            proj23 = chain(proj_mm_units(2, st2), proj_mm_units(3, st3))
            for _ in range(5):
                next(half0s, None)
            weave(half0s, proj23, 1)

        def final_units(lo, hi):
            for sb in range(lo, hi):
                emit_final(sb)
                yield

        half1s_012 = chain(*[attn_half(hl, 1) for hl in (1, 3, 0)])
        weave(half1s_012, final_units(0, 7), 7)
        if mode == "causal":
            # the half1 stretch is exp-bound on ScalarE: finals are the
            # only ScalarE-free PE work, so spread them across all four
            # heads.  Finals 8..11 follow region-2's norm (~step 12.5);
            # 12..15 trail region 3 with casts split DVE/ScalarE.
            def tail_finals():
                yield emit_final(7)
                for sb in range(8, 12):
                    emit_final(sb)
                    yield
            fin = tail_finals()
            for i, _ in enumerate(attn_half(2, 1, tail_norm=True)):
                if i == 3 or i >= 13:
                    next(fin, None)
            for _ in fin:
                pass
            emit_final(12, copy_engs=(nc.vector, nc.scalar))
            emit_final(13, copy_engs=(nc.scalar, nc.vector))
            emit_final(14, store_eng=nc.scalar, copy_engs=(nc.vector, nc.scalar))
            emit_final(15, store_eng=nc.scalar, copy_engs=(nc.scalar, nc.vector))
        else:
            f811 = final_units(7, 12)
            for i, _ in enumerate(attn_half(2, 1)):
                if i >= 16 and (i - 16) % 2 == 0:
                    next(f811, None)
            for _ in f811:
                pass
            for sb in range(12, 16):
                emit_final(sb)

    nc.compile()
    return nc


_ONES1 = np.ones((1, 512), dtype=np.float32)

_CACHE = {}


def _get_nc(mode, bias_k, bias_v):
    key = (mode, bias_k, bias_v)
    if key not in _CACHE:
        _CACHE[key] = _build(mode, bias_k, bias_v)
    return _CACHE[key]


def make_in_maps(q, k, v, mask, Wk, bk, Wv, bv, Wo, bo):
    """Host-side sharding. Returns (mode, bias flags, in_maps)."""
    import ml_dtypes

    bf16 = ml_dtypes.bfloat16
    q = np.asarray(q, dtype=np.float32)
    k = np.asarray(k, dtype=np.float32)
    v = np.asarray(v, dtype=np.float32)
    Wk = np.asarray(Wk, dtype=np.float32).astype(bf16)
    Wv = np.asarray(Wv, dtype=np.float32).astype(bf16)
    Wo = np.asarray(Wo, dtype=np.float32).astype(bf16)
    bk = np.asarray(bk, dtype=np.float32).reshape(-1)
    bv = np.asarray(bv, dtype=np.float32).reshape(-1)
    bo = np.asarray(bo, dtype=np.float32).reshape(-1)
    mask2d = np.asarray(mask, dtype=np.float32).reshape(S, S)

    if not mask2d.any():
        mode = "none"
    elif np.array_equal(mask2d, np.triu(np.ones((S, S), np.float32), 1)):
        mode = "causal"
    else:
        mode = "general"
    bias_k, bias_v, bias_o = bool(bk.any()), bool(bv.any()), bool(bo.any())

    def tile_xT(x):
        # [S, D] -> [NCH*128, 8*512]: row c*128+p, col dc*512+s holds
        # x[c*512+s, dc*128+p] (= xT[dc*128+p, c*512+s])
        t = x.reshape(NCH, 512, 8, 128).transpose(0, 3, 2, 1)
        return np.ascontiguousarray(t.reshape(NCH * 128, 8 * 512)).astype(bf16)

    qT = [tile_xT(q[b]) for b in range(B)]
    kT = [tile_xT(k[b]) for b in range(B)]
    vT = [tile_xT(v[b]) for b in range(B)]
    if mode == "general":
        # pre-scale so adding before the fused exp scale matches the
        # reference's post-scale add:  (raw + m)*S_INV == raw*S_INV + mask*(-1e9)
        maskT = np.ascontiguousarray(
            (mask2d.T * np.float32(-1e9 / S_INV)).astype(bf16))

    in_maps = []
    for core in range(NCORES):
        b, g = divmod(core, HPC)
        cs = slice(CW * g, CW * (g + 1))
        im = {
            "qT": qT[b], "kT": kT[b], "vT": vT[b],
            "wk": np.ascontiguousarray(Wk[:, cs]),
            "wv": np.ascontiguousarray(Wv[:, cs]),
            "wo": np.ascontiguousarray(Wo[cs, :]),
        }
        if bias_k or bias_v:
            im["ones1"] = _ONES1.astype(bf16)
        if bias_k:
            im["bk"] = np.ascontiguousarray(bk[cs]).astype(bf16).reshape(1, CW)
        if bias_v:
            im["bv"] = np.ascontiguousarray(bv[cs]).astype(bf16).reshape(1, CW)
        if mode == "general":
            im["maskT"] = maskT
        in_maps.append(im)
    return mode, (bias_k, bias_v, bias_o), in_maps


def assemble(results, bo=None):
    """Sum per-core partial outputs into the full [B, S, D] output."""
    full = np.zeros((B, S, D), dtype=np.float32)
    for b in range(B):
        acc = results[4 * b]["out"].astype(np.float32)
        for c in range(4 * b + 1, 4 * b + 4):
            acc = acc + results[c]["out"].astype(np.float32)
        if bo is not None:
            acc = acc + bo
        full[b] = acc
    return full


def kernel(q, k, v, mask, Wk, bk, Wv, bv, Wo, bo):
    mode, (bias_k, bias_v, bias_o), in_maps = make_in_maps(
        q, k, v, mask, Wk, bk, Wv, bv, Wo, bo)
    nc = _get_nc(mode, bias_k, bias_v)
    res = bass_utils.run_bass_kernel_spmd(nc, in_maps, core_ids=list(range(NCORES)))
    bo_arr = np.asarray(bo, dtype=np.float32).reshape(-1) if bias_o else None
    return assemble(res.results, bo_arr)


# revision 53
# speedup vs baseline: 1.0439x; 1.0439x over previous
"""Multi-head attention (B=2, S=2048, D=1024, H=16) on 8 Trainium2 cores.

Sharding: data-parallel over the 2 batches x tensor-parallel over 4 groups
of 4 heads.  Core c handles batch c//4 and heads [4*(c%4) : 4*(c%4)+4]
(columns [256*(c%4) : +256] of Wk/Wv, same rows of Wo).  Each core produces
a partial [S, D] output (its heads' contribution to o @ Wo); the host sums
the 4 partials per batch (and adds bo once).

Per-core dataflow (bf16 everywhere; fp32 PSUM accumulation):
  qT,kT,vT [D,S] arrive pre-transposed AND pre-cast to bf16 on the host, so
  all loads ride the fast HWDGE queues at half the bytes.  Projections
  produce QT,KT [128,2,S] (head-major rows) and V [sk,hd] with an extra
  ones column.  Attention per head in "scores-transposed" layout
  [sk_part, sq_free]: scoresT = KT_j^T @ QT (one matmul per k-block row);
  exp on ScalarE (scale folded in; no max subtraction - scores are O(6));
  the causal diagonal block is handled by zeroing the upper triangle of the
  exp tile with one DVE multiply (cheaper than the identity-matmul + mask
  trick: no PE work, no ident reloads).  UT[65, S] += Vaug_j^T @ expT
  accumulated in PSUM, row 64 = softmax denominators (from the ones
  column).  Normalization is region-wise (512 cols at a time, as soon as
  that region's last k-block lands): sums -> DMA reshape [1,512]->[128,4]
  -> DVE reciprocal -> DMA back -> gpsimd partition_broadcast -> one DVE
  multiply into oT [d_part, sq].  Final: out = oT^T @ Wo per 128-row
  block, bf16 DMA to HBM (host sums partials in fp32).

A few throwaway warm-up matmuls run during the initial DMA fill so the PE
HAM clock-gate is already at 2.4 GHz when the first projection issues.
"""

from contextlib import ExitStack

import numpy as np

import concourse.bass as bass
import concourse.tile as tile
from concourse import bacc, bass_utils, mybir
from concourse.masks import make_identity

B, S, D, H = 2, 2048, 1024, 16
HD = D // H            # 64
NCORES = 8
HPC = 4                # heads per core
CW = HPC * HD          # 256 weight cols per core
NCH = 4                # sequence chunks of 512
S_INV = float(1.0 / (np.sqrt(np.float32(HD)) + np.float32(1e-8)))

F32 = mybir.dt.float32
BF16 = mybir.dt.bfloat16


def _build(mode: str, bias_k: bool, bias_v: bool):
    """Build + compile the SPMD program.  mode: 'causal' | 'none' | 'general'"""
    nc = bacc.Bacc("TRN2", target_bir_lowering=False, debug=False,
                   num_devices=NCORES)

    # q/k/v arrive host-pre-tiled as [NCH*128, 8*512]: row c*128+p, col
    # dc*512+s holds xT[dc*128+p, c*512+s].  One chunk = one DMA with 8KB
    # contiguous per-partition lines (vs 1KB slicing [D,S] directly).
    qT_d = nc.dram_tensor("qT", [NCH * 128, 8 * 512], BF16, kind="ExternalInput").ap()
    kT_d = nc.dram_tensor("kT", [NCH * 128, 8 * 512], BF16, kind="ExternalInput").ap()
    vT_d = nc.dram_tensor("vT", [NCH * 128, 8 * 512], BF16, kind="ExternalInput").ap()
    wk_d = nc.dram_tensor("wk", [D, CW], BF16, kind="ExternalInput").ap()
    wv_d = nc.dram_tensor("wv", [D, CW], BF16, kind="ExternalInput").ap()
    wo_d = nc.dram_tensor("wo", [CW, D], BF16, kind="ExternalInput").ap()
    bk_d = nc.dram_tensor("bk", [1, CW], BF16, kind="ExternalInput").ap() if bias_k else None
    bv_d = nc.dram_tensor("bv", [1, CW], BF16, kind="ExternalInput").ap() if bias_v else None
    maskT_d = (nc.dram_tensor("maskT", [S, S], BF16, kind="ExternalInput").ap()
               if mode == "general" else None)
    ones1_d = (nc.dram_tensor("ones1", [1, 512], BF16, kind="ExternalInput").ap()
               if (bias_k or bias_v) else None)
    out_d = nc.dram_tensor("out", [S, D], BF16, kind="ExternalOutput").ap()

    with tile.TileContext(nc) as tc, ExitStack() as ctx:
        sb1 = ctx.enter_context(tc.tile_pool(name="persist", bufs=1))
        v_pool = ctx.enter_context(tc.tile_pool(name="v", bufs=NCH))
        stage_pool = ctx.enter_context(tc.tile_pool(name="stage", bufs=6))
        exp_pool = ctx.enter_context(tc.tile_pool(name="exp", bufs=4))
        sums_pool = ctx.enter_context(tc.tile_pool(name="sums", bufs=4))
        srt_pool = ctx.enter_context(tc.tile_pool(name="srt", bufs=4))
        rcb_pool = ctx.enter_context(tc.tile_pool(name="rcb", bufs=4))
        bc_pool = ctx.enter_context(tc.tile_pool(name="bc", bufs=5))
        u_pool = ctx.enter_context(tc.tile_pool(name="u", bufs=4))
        ottmp_pool = ctx.enter_context(tc.tile_pool(name="ottmp", bufs=2))
        outsb_pool = ctx.enter_context(tc.tile_pool(name="outsb", bufs=4))
        sc_pool = ctx.enter_context(tc.tile_pool(name="sc", bufs=3, space="PSUM"))
        ut_pool = ctx.enter_context(tc.tile_pool(name="ut", bufs=1, space="PSUM"))
        if mode == "general":
            mask_pool = ctx.enter_context(tc.tile_pool(name="mask", bufs=3))

        # ---- PE warm-up: keep HAM busy while the first inputs stream in.
        # The seed memset rides DVE: GpSimd's wake-up path would hold the
        # first matmul until ~10us.  Results are discarded.
        wm = sb1.tile([128, 512], BF16)
        nc.vector.memset(wm[:], 0.0)
        wps = sc_pool.tile([128, 512], F32, tag="sc", name="warm")
        NWARM = 20
        for i in range(NWARM):
            nc.tensor.matmul(wps[:], wm[:, 0:128], wm[:],
                             start=(i == 0), stop=(i == NWARM - 1))

        # ---- constants / weights -------------------------------------
        wk_sb = sb1.tile([128, 8, CW], BF16)
        wv_sb = sb1.tile([128, 8, CW], BF16)
        nc.sync.dma_start(wk_sb[:], wk_d.rearrange("(c p) n -> p c n", p=128))
        nc.sync.dma_start(wv_sb[:], wv_d.rearrange("(c p) n -> p c n", p=128))
        wo_sb = sb1.tile([128, 2, D], BF16)  # loaded late, after staging
        if bias_k:
            bk_sb = sb1.tile([1, CW], BF16)
            nc.sync.dma_start(bk_sb[:], bk_d[:])
        if bias_v:
            bv_sb = sb1.tile([1, CW], BF16)
            nc.sync.dma_start(bv_sb[:], bv_d[:])
        if bias_k or bias_v:
            ones_sb = sb1.tile([1, 512], BF16)
            nc.sync.dma_start(ones_sb[:], ones1_d[:])
        if mode == "general":
            ident = sb1.tile([128, 128], BF16)
            make_identity(nc, ident[:])
        if mode == "causal":
            # trimask[p, f] = 1 where f >= p (sq >= sk), else 0
            trimask = sb1.tile([128, 128], BF16)
            nc.gpsimd.memset(trimask[:], 1.0)
            nc.gpsimd.affine_select(
                out=trimask[:], in_=trimask[:],
                compare_op=mybir.AluOpType.is_ge,
                fill=0.0, base=0,
                pattern=[[1, 128]], channel_multiplier=-1,
            )

        # V tiles: [128 sk, 4 blk, 4 head, 66] - col 64 is the ones column
        v_tiles = [v_pool.tile([128, 4, HPC, 66], BF16, tag="v", name=f"v{c}")
                   for c in range(NCH)]
        for c in range(NCH):
            nc.gpsimd.memset(v_tiles[c][:, :, :, 64:65], 1.0)
        qt_sb = sb1.tile([128, 2, S], BF16)
        kt_sb = sb1.tile([128, 2, S], BF16)
        oT_sb = sb1.tile([128, 2, S], BF16)

        def ps_copy(dst, src):
            nc.vector.tensor_copy(dst, src)

        # ---- phase 1: projections (helpers) --------------------------
        def load_stage(c, nm, td, split=False):
            stg = stage_pool.tile([128, 8, 512], BF16, tag="stage",
                                  name=f"{nm}st{c}")
            src = td[bass.ds(128 * c, 128), :].rearrange(
                "p (dc s) -> p dc s", dc=8)
            if split:
                # split so the consumer can begin on the first half
                nc.sync.dma_start(stg[:, 0:4, :], src[:, 0:4, :])
                nc.sync.dma_start(stg[:, 4:8, :], src[:, 4:8, :])
            else:
                nc.sync.dma_start(stg[:], src)
            return stg

        def emit_proj_loads(c):
            # c >= 2 only; load in the order projections consume
            return [load_stage(c, nm, td) for nm, td in
                    (("q", qT_d), ("k", kT_d), ("v", vT_d))]

        def proj_mm_units(c, stages, part="both"):
            """Yield once per PSUM accumulation group (small PE work unit)."""
            if c >= 2:
                (qst, kst, vst) = stages
            else:
                (kst, qst, vst) = stages
            do_kq = part in ("both", "kq")
            do_v = part in ("both", "v")
            # KT / QT projections (transposed layout, 2 m-halves of 128)
            kq_order = ((((qst, qt_sb), (kst, kt_sb)) if c >= 2 else
                        ((kst, kt_sb), (qst, qt_sb))) if do_kq else ())
            for ti, (st, dst) in enumerate(kq_order):
                for m in range(2):
                    ps = sc_pool.tile([128, 512], F32, tag="sc", name=f"psp{c}_{ti}_{m}")
                    first = True
                    if bias_k:
                        nc.tensor.matmul(ps[:], bk_sb[0:1, bass.ds(m * 128, 128)],
                                         ones_sb[0:1, :], start=True, stop=False)
                        first = False
                    for dc in range(8):
                        nc.tensor.matmul(
                            ps[:],
                            wk_sb[:, dc, bass.ds(m * 128, 128)],
                            st[:, dc, :],
                            start=first, stop=(dc == 7))
                        first = False
                    ps_copy(dst[:, m, bass.ds(c * 512, 512)], ps[:])
                    yield
            # V projection (natural layout)
            for half in range(2) if do_v else ():
                psv = sc_pool.tile([128, 512], F32, tag="sc", name=f"psv{c}_{half}")
                for loc in range(2):
                    blk = 2 * half + loc
                    reg = psv[:, bass.ds(loc * 256, 256)]
                    first = True
                    if bias_v:
                        nc.tensor.matmul(reg, ones_sb[0:1, 0:128], bv_sb[0:1, :],
                                         start=True, stop=False)
                        first = False
                    for dc in range(8):
                        nc.tensor.matmul(
                            reg,
                            vst[:, dc, bass.ds(blk * 128, 128)],
                            wv_sb[:, dc, :],
                            start=first, stop=(dc == 7))
                        first = False
                ps_copy(v_tiles[c][:, bass.ds(2 * half, 2), :, 0:64],
                        psv[:].rearrange("p (b h e) -> p b h e", b=2, h=HPC))
                yield

        # ---- phase 2: attention, one (head, sq-half) pass ------------
        full_grid = mode != "causal"

        def attn_half(hl, half, tail_norm=False):
            m = hl // 2
            p0 = 64 * (hl % 2)
            base = 1024 * half
            regions = (2 * half, 2 * half + 1)
            ut = ut_pool.tile([128, 1024], F32, tag="ut", name=f"ut{hl}_{half}")

            if full_grid:
                steps = list(range(16))
                last_j = {r: 15 for r in regions}
            else:
                steps = list(range(8 * half + 8))
                last_j = {r: 4 * r + 3 for r in regions}

            win_ps = {}
            win_exp = {}

            def active(j):
                """absolute start col of k-block j's active window portion."""
                return base if full_grid else max(128 * j, base)

            def emit_scores(j):
                ps = sc_pool.tile([128, 1024], F32, tag="sc",
                                  name=f"sc{hl}_{half}_{j}")
                win_ps[j] = ps
                a0 = active(j)
                lhsT = kt_sb[p0:p0 + 64, m, bass.ds(128 * j, 128)]
                if mode == "causal":
                    for s in range(2):
                        lo, hi = base + 512 * s, base + 512 * s + 512
                        if hi <= a0:
                            continue
                        nlo = max(lo, a0)
                        nc.tensor.matmul(
                            ps[:, bass.ds(nlo - base, hi - nlo)],
                            lhsT,
                            qt_sb[p0:p0 + 64, m, bass.ds(nlo, hi - nlo)],
                            start=True, stop=True)
                    return
                if mode == "general":
                    mt = mask_pool.tile([128, 1024], BF16, tag="mask",
                                        name=f"mt{hl}_{half}_{j}")
                    nc.sync.dma_start(
                        mt[:],
                        maskT_d[bass.ds(128 * j, 128), bass.ds(base, 1024)])
                for s in range(2):
                    lo = base + 512 * s
                    reg = ps[:, bass.ds(lo - base, 512)]
                    rhs = qt_sb[p0:p0 + 64, m, bass.ds(lo, 512)]
                    nc.tensor.matmul(reg, lhsT, rhs, start=True,
                                     stop=(mode != "general"))
                    if mode == "general":
                        nc.tensor.matmul(reg, ident[:],
                                         mt[:, bass.ds(lo - base, 512)],
                                         start=False, stop=True)

            def emit_exp(j):
                ps = win_ps[j]
                off = active(j) - base
                et = exp_pool.tile([128, 1024], BF16, tag="exp",
                                   name=f"e{hl}_{half}_{j}")
                win_exp[j] = et
                nc.scalar.activation(et[:, off:1024], ps[:, off:1024],
                                     mybir.ActivationFunctionType.Exp, scale=S_INV)
                if mode == "causal" and 128 * j >= base:
                    # zero the upper triangle (sq < sk) of the diagonal block
                    # (DVE: GpSimd's ~1us semaphore-wait latency poisons the
                    # exp->PV chain)
                    d0 = 128 * j - base
                    nc.vector.tensor_mul(et[:, bass.ds(d0, 128)],
                                         et[:, bass.ds(d0, 128)], trimask[:])

            def emit_pv(j):
                et = win_exp.pop(j)
                win_ps.pop(j)
                a0 = active(j)
                for s in range(2):
                    lo, hi = base + 512 * s, base + 512 * s + 512
                    if hi <= a0:
                        continue
                    nlo = max(lo, a0)
                    r = nlo // 512
                    nc.tensor.matmul(
                        ut[0:65, bass.ds(nlo - base, hi - nlo)],
                        v_tiles[j // 4][:, j % 4, hl, 0:65],
                        et[:, bass.ds(nlo - base, hi - nlo)],
                        start=(j == 0), stop=(j == last_j[r]))

            if p0 == 0:
                dst = oT_sb[0:64, m, bass.ds(base, 1024)]
                ott = None
            else:
                ott = ottmp_pool.tile([64, 1024], BF16, tag="ottmp",
                                      name=f"ott{hl}_{half}")
                dst = ott[:, :]

            def emit_norm(r):
                """copy U+sums out of PSUM, then recip -> bcast -> multiply."""
                # the very last region's small DMAs ride the scalar ring:
                # the sync ring is busy draining output stores by then
                dma_eng = (nc.scalar if (tail_norm and r == regions[1])
                           else nc.sync)
                u = u_pool.tile([65, 512], F32, tag="u", name=f"u{hl}_{r}")
                nc.vector.tensor_copy(u[:], ut[0:65, bass.ds(512 * r - base, 512)])
                srt = srt_pool.tile([128, 4], F32, tag="srt", name=f"srt{hl}_{r}")
                dma_eng.dma_start(srt[:], u[64:65, :])
                nc.vector.reciprocal(srt[:], srt[:])
                rcb = rcb_pool.tile([1, 512], F32, tag="rcb", name=f"rcb{hl}_{r}")
                dma_eng.dma_start(rcb[0:1, :], srt[:])
                bc = bc_pool.tile([64, 512], F32, tag="bc", name=f"bc{hl}_{r}")
                nc.gpsimd.partition_broadcast(bc[:], rcb[:], channels=64)
                nc.vector.tensor_mul(
                    dst[:, bass.ds(512 * r - base, 512)],
                    u[0:64, :],
                    bc[:, :])
                if p0:
                    nc.sync.dma_start(
                        oT_sb[64:128, m, bass.ds(512 * r, 512)],
                        ott[:, bass.ds(512 * r - base, 512)])

            LOOKAHEAD = 2
            for i in range(min(LOOKAHEAD, len(steps))):
                emit_scores(steps[i])
            for i, j in enumerate(steps):
                if i + LOOKAHEAD < len(steps):
                    emit_scores(steps[i + LOOKAHEAD])
                emit_exp(j)
                emit_pv(j)
                for r in regions:
                    if j == last_j[r]:
                        emit_norm(r)
                yield

        def emit_final(sb, store_eng=None, copy_engs=None):
            ob = outsb_pool.tile([128, D], BF16, tag="outsb", name=f"ob{sb}")
            for nh in range(2):
                ps = sc_pool.tile([128, 512], F32, tag="sc", name=f"pso{sb}_{nh}")
                for mm_ in range(2):
                    nc.tensor.matmul(
                        ps[:],
                        oT_sb[:, mm_, bass.ds(sb * 128, 128)],
                        wo_sb[:, mm_, bass.ds(nh * 512, 512)],
                        start=(mm_ == 0), stop=(mm_ == 1))
                eng = (copy_engs or (nc.vector, nc.vector))[nh]
                if eng is nc.scalar:
                    nc.scalar.copy(ob[:, bass.ds(nh * 512, 512)], ps[:])
                else:
                    eng.tensor_copy(ob[:, bass.ds(nh * 512, 512)], ps[:])
            (store_eng or nc.sync).dma_start(out_d[bass.ds(sb * 128, 128), :], ob[:])

        def drain(gen):
            for _ in gen:
                pass

        def weave(step_gen, unit_gen, steps_per_unit):
            """Emit attention steps, inserting one PE-heavy unit every N."""
            i = 0
            for _ in step_gen:
                i += 1
                if i % steps_per_unit == 0:
                    next(unit_gen, None)
            for _ in unit_gen:
                pass

        def chain(*gens):
            for g in gens:
                yield from g

        # ---- orchestration: overlap proj DMA with attention ----------
        # chunk-0 k/q split + chunk-1 k hoisted before chunk-0 v: loads
        # land in the exact order projection 0/1 consume them.
        k0 = load_stage(0, "k", kT_d, split=True)
        q0 = load_stage(0, "q", qT_d, split=True)
        k1 = load_stage(1, "k", kT_d)
        v0 = load_stage(0, "v", vT_d)
        q1 = load_stage(1, "q", qT_d)
        v1 = load_stage(1, "v", vT_d)
        st0, st1 = (k0, q0, v0), (k1, q1, v1)
        drain(proj_mm_units(0, st0))
        st2 = emit_proj_loads(2)
        if full_grid:
            drain(proj_mm_units(1, st1))
            st3 = emit_proj_loads(3)
            nc.sync.dma_start(wo_sb[:], wo_d.rearrange("(m p) n -> p m n", p=128))
            # every k-block needs every chunk: project everything first
            drain(proj_mm_units(2, st2))
            drain(proj_mm_units(3, st3))
            for hl in (1, 3, 0, 2):
                drain(attn_half(hl, 0))
        else:
            drain(proj_mm_units(1, st1, part="kq"))
            half0s = chain(*[attn_half(hl, 0) for hl in (1, 3, 0, 2)])
            # a few attention steps cover v-chunk-1's DMA landing window
            # so its projection doesn't head-of-line-block the PE
            for _ in range(3):
                next(half0s, None)
            drain(proj_mm_units(1, st1, part="v"))
            st3 = emit_proj_loads(3)
            nc.sync.dma_start(wo_sb[:], wo_d.rearrange("(m p) n -> p m n", p=128))
            proj23 = chain(proj_mm_units(2, st2), proj_mm_units(3, st3))
            for _ in range(5):
                next(half0s, None)
            weave(half0s, proj23, 1)

        def final_units(lo, hi):
            for sb in range(lo, hi):
                emit_final(sb)
                yield

        half1s_012 = chain(*[attn_half(hl, 1) for hl in (1, 3, 0)])
        weave(half1s_012, final_units(0, 7), 7)
        if mode == "causal":
            # the half1 stretch is exp-bound on ScalarE: finals are the
            # only ScalarE-free PE work, so spread them across all four
            # heads.  Finals 8..11 follow region-2's norm (~step 12.5);
            # 12..15 trail region 3 with casts split DVE/ScalarE.
            def tail_finals():
                yield emit_final(7)
                for sb in range(8, 12):
                    emit_final(sb)
                    yield
            fin = tail_finals()
            for i, _ in enumerate(attn_half(2, 1)):
                if i == 3 or i >= 13:
                    next(fin, None)
            for _ in fin:
                pass
            emit_final(12, copy_engs=(nc.vector, nc.scalar))
            emit_final(13, copy_engs=(nc.scalar, nc.vector))
            emit_final(14, store_eng=nc.scalar, copy_engs=(nc.vector, nc.scalar))
            emit_final(15, store_eng=nc.scalar, copy_engs=(nc.scalar, nc.vector))
        else:
            f811 = final_units(7, 12)
            for i, _ in enumerate(attn_half(2, 1)):
                if i >= 16 and (i - 16) % 2 == 0:
                    next(f811, None)
            for _ in f811:
                pass
            for sb in range(12, 16):
                emit_final(sb)

    nc.compile()
    return nc


_ONES1 = np.ones((1, 512), dtype=np.float32)

_CACHE = {}


def _get_nc(mode, bias_k, bias_v):
    key = (mode, bias_k, bias_v)
    if key not in _CACHE:
        _CACHE[key] = _build(mode, bias_k, bias_v)
    return _CACHE[key]


def make_in_maps(q, k, v, mask, Wk, bk, Wv, bv, Wo, bo):
    """Host-side sharding. Returns (mode, bias flags, in_maps)."""
    import ml_dtypes

    bf16 = ml_dtypes.bfloat16
    q = np.asarray(q, dtype=np.float32)
    k = np.asarray(k, dtype=np.float32)
    v = np.asarray(v, dtype=np.float32)
    Wk = np.asarray(Wk, dtype=np.float32).astype(bf16)
    Wv = np.asarray(Wv, dtype=np.float32).astype(bf16)
    Wo = np.asarray(Wo, dtype=np.float32).astype(bf16)
    bk = np.asarray(bk, dtype=np.float32).reshape(-1)
    bv = np.asarray(bv, dtype=np.float32).reshape(-1)
    bo = np.asarray(bo, dtype=np.float32).reshape(-1)
    mask2d = np.asarray(mask, dtype=np.float32).reshape(S, S)

    if not mask2d.any():
        mode = "none"
    elif np.array_equal(mask2d, np.triu(np.ones((S, S), np.float32), 1)):
        mode = "causal"
    else:
        mode = "general"
    bias_k, bias_v, bias_o = bool(bk.any()), bool(bv.any()), bool(bo.any())

    def tile_xT(x):
        # [S, D] -> [NCH*128, 8*512]: row c*128+p, col dc*512+s holds
        # x[c*512+s, dc*128+p] (= xT[dc*128+p, c*512+s])
        t = x.reshape(NCH, 512, 8, 128).transpose(0, 3, 2, 1)
        return np.ascontiguousarray(t.reshape(NCH * 128, 8 * 512)).astype(bf16)

    qT = [tile_xT(q[b]) for b in range(B)]
    kT = [tile_xT(k[b]) for b in range(B)]
    vT = [tile_xT(v[b]) for b in range(B)]
    if mode == "general":
        # pre-scale so adding before the fused exp scale matches the
        # reference's post-scale add:  (raw + m)*S_INV == raw*S_INV + mask*(-1e9)
        maskT = np.ascontiguousarray(
            (mask2d.T * np.float32(-1e9 / S_INV)).astype(bf16))

    in_maps = []
    for core in range(NCORES):
        b, g = divmod(core, HPC)
        cs = slice(CW * g, CW * (g + 1))
        im = {
            "qT": qT[b], "kT": kT[b], "vT": vT[b],
            "wk": np.ascontiguousarray(Wk[:, cs]),
            "wv": np.ascontiguousarray(Wv[:, cs]),
            "wo": np.ascontiguousarray(Wo[cs, :]),
        }
        if bias_k or bias_v:
            im["ones1"] = _ONES1.astype(bf16)
        if bias_k:
            im["bk"] = np.ascontiguousarray(bk[cs]).astype(bf16).reshape(1, CW)
        if bias_v:
            im["bv"] = np.ascontiguousarray(bv[cs]).astype(bf16).reshape(1, CW)
        if mode == "general":
            im["maskT"] = maskT
        in_maps.append(im)
    return mode, (bias_k, bias_v, bias_o), in_maps


def assemble(results, bo=None):
    """Sum per-core partial outputs into the full [B, S, D] output."""
    full = np.zeros((B, S, D), dtype=np.float32)
    for b in range(B):
        acc = results[4 * b]["out"].astype(np.float32)
        for c in range(4 * b + 1, 4 * b + 4):
            acc = acc + results[c]["out"].astype(np.float32)
        if bo is not None:
            acc = acc + bo
        full[b] = acc
    return full


def kernel(q, k, v, mask, Wk, bk, Wv, bv, Wo, bo):
    mode, (bias_k, bias_v, bias_o), in_maps = make_in_maps(
        q, k, v, mask, Wk, bk, Wv, bv, Wo, bo)
    nc = _get_nc(mode, bias_k, bias_v)
    res = bass_utils.run_bass_kernel_spmd(nc, in_maps, core_ids=list(range(NCORES)))
    bo_arr = np.asarray(bo, dtype=np.float32).reshape(-1) if bias_o else None
    return assemble(res.results, bo_arr)
